# revision 5
# baseline (speedup 1.0000x reference)
"""Baichuan attention on 8 Trainium2 NeuronCores — tensor-parallel over heads.

Sharding: core c computes heads [4c, 4c+4): its slice of the fused QKV
projection, attention for those heads, then 1/8 of o_proj's output columns
after an AllGather of the per-core context slices (moves 4MB/rank instead of
a 32MB AllReduce of partial sums; mathematically identical to the module's
world_size logic).

Device layout is fully transposed: scores are computed as scoresT[k, q]
blocks so the PE contraction dim (head_dim / s_k) always sits on SBUF
partitions and every matmul has a 512-wide moving operand. Matmul operands
are carried as float32r end-to-end (PE fast path: 1 cyc/row vs 4 for true
fp32); accumulation stays fp32 in PSUM.
"""

import numpy as np

import concourse.bacc as bacc
import concourse.mybir as mybir
import concourse.tile as tile
from concourse.bass_utils import run_bass_kernel_spmd
from concourse.masks import make_identity

F32 = mybir.dt.float32
F32R = mybir.dt.float32r

N_CORES = 8
NUM_HEADS = 32
HEAD_DIM = 128
P = 128          # SBUF partitions / PE contraction tile
SQ = 512         # s_q block width (PSUM bank = 512 fp32)
USE_F32R = True

_CACHE: dict = {}


def build(S, H, block_cls, use_f32r=USE_F32R):
    """Build the SPMD program. block_cls[(t, b)] = 'plain' | 'mask' for every
    computed scoresT block ([128 s_k] x [SQ s_q]); absent = fully masked, skip.
    """
    MD = F32R if use_f32r else F32      # matmul operand dtype
    hpc = NUM_HEADS // N_CORES          # heads per core
    dpc = hpc * HEAD_DIM                # per-core slice of the hidden dim
    n_ht = H // P                       # contraction tiles for QKV/o_proj
    n_ot = 3 * dpc // P                 # per-core QKV output tiles
    n_sq = S // SQ                      # s_q blocks
    n_st = S // P                       # s_k tiles
    s_half = S // 2
    sb_per_half = s_half // SQ
    scale = 1.0 / np.sqrt(np.float32(HEAD_DIM))

    nc = bacc.Bacc("TRN2", target_bir_lowering=False, debug=False,
                   num_devices=N_CORES)

    xT = nc.dram_tensor("xT", [H, S], MD, kind="ExternalInput")
    wqkvT = nc.dram_tensor("wqkvT", [H, 3 * dpc], MD, kind="ExternalInput")
    maskT = nc.dram_tensor("maskT", [S, S], F32, kind="ExternalInput")
    woT = nc.dram_tensor("woT", [H, dpc], MD, kind="ExternalInput")
    out_cols = nc.dram_tensor("out_cols", [S, dpc], F32, kind="ExternalOutput")

    qkvT_s = nc.dram_tensor("qkvT_s", [3 * dpc, S], MD)
    gat_in = nc.dram_tensor("gat_in", [dpc, S], MD)
    ct = nc.dram_tensor("ct", [H, S], MD, addr_space="Shared")

    xT_t = xT.ap().rearrange("(t p) s -> p t s", p=P)
    wqkvT_t = wqkvT.ap().rearrange("(t p) o -> p t o", p=P)
    woT_t = woT.ap().rearrange("(t p) j -> p t j", p=P)
    ct_t = ct.ap().rearrange("(t p) s -> p t s", p=P)

    # which mask blocks are needed (shared across heads)
    mask_blocks = sorted({k for k, v in block_cls.items() if v == "mask"})
    mask_slot = {k: i for i, k in enumerate(mask_blocks)}

    with tile.TileContext(nc) as tc:
        # ---- constants ----
        with tc.tile_pool(name="consts", bufs=1) as cpool:
            ident_f = cpool.tile([P, P], F32, tag="ident_f")
            make_identity(nc, ident_f[:])
            ident = cpool.tile([P, P], MD, tag="ident")
            nc.scalar.copy(ident[:], ident_f[:])
            ones_f = cpool.tile([P, P], F32, tag="ones_f")
            nc.gpsimd.memset(ones_f[:], 1.0)
            ones_sq = cpool.tile([P, P], MD, tag="ones_sq")
            nc.scalar.copy(ones_sq[:], ones_f[:])

            # =============== phase 1: QKV projection ===============
            with (
                tc.tile_pool(name="qkv_sb", bufs=1) as xpool,
                tc.tile_pool(name="qkv_w", bufs=3) as wpool,
                tc.tile_pool(name="qkv_stage", bufs=4) as spool,
                tc.tile_pool(name="qkv_ps", bufs=4, space="PSUM") as pspool,
            ):
                for half in range(2):
                    x_tile = xpool.tile([P, n_ht, s_half], MD, tag="x")
                    for t in range(n_ht):
                        nc.sync.dma_start(
                            x_tile[:, t, :],
                            xT_t[:, t, half * s_half:(half + 1) * s_half])
                    for ot in range(n_ot):
                        w_tile = wpool.tile([P, n_ht, P], MD, tag="w")
                        nc.sync.dma_start(
                            w_tile[:], wqkvT_t[:, :, ot * P:(ot + 1) * P])
                        for sb in range(sb_per_half):
                            ps = pspool.tile([P, SQ], F32, tag="qkv")
                            for t in range(n_ht):
                                nc.tensor.matmul(
                                    ps[:],
                                    w_tile[:, t, :],
                                    x_tile[:, t, sb * SQ:(sb + 1) * SQ],
                                    start=(t == 0), stop=(t == n_ht - 1))
                            st = spool.tile([P, SQ], MD, tag="stage")
                            # fold the softmax scale into q at eviction
                            mul = scale if ot < dpc // P else 1.0
                            nc.scalar.mul(st[:], ps[:], mul)
                            nc.sync.dma_start(
                                qkvT_s.ap()[ot * P:(ot + 1) * P,
                                            half * s_half + sb * SQ:
                                            half * s_half + (sb + 1) * SQ],
                                st[:])

            # =============== phase 2: attention per head ===============
            with (
                tc.tile_pool(name="at_mask", bufs=1) as mpool,
                tc.tile_pool(name="at_qkv", bufs=2) as hpool,
                tc.tile_pool(name="at_v", bufs=2) as vpool,
                tc.tile_pool(name="at_exp", bufs=3) as epool,
                tc.tile_pool(name="at_out", bufs=3) as opool,
                tc.tile_pool(name="at_r", bufs=2) as rpool,
                tc.tile_pool(name="at_ps", bufs=2, space="PSUM") as aps,
                tc.tile_pool(name="at_ps1", bufs=1, space="PSUM") as aps1,
            ):
                if mask_blocks:
                    mtile = mpool.tile([P, len(mask_blocks), SQ], F32,
                                       tag="mask")
                    for (t, b), i in mask_slot.items():
                        nc.sync.dma_start(
                            mtile[:, i, :],
                            maskT.ap()[t * P:(t + 1) * P,
                                       b * SQ:(b + 1) * SQ])

                for h in range(hpc):
                    qT = hpool.tile([P, S], MD, tag="qT")
                    kT = hpool.tile([P, S], MD, tag="kT")
                    vT = hpool.tile([P, S], MD, tag="vT")
                    nc.sync.dma_start(qT[:], qkvT_s.ap()[h * P:(h + 1) * P, :])
                    nc.sync.dma_start(
                        kT[:], qkvT_s.ap()[dpc + h * P:dpc + (h + 1) * P, :])
                    nc.sync.dma_start(
                        vT[:], qkvT_s.ap()[2 * dpc + h * P:
                                           2 * dpc + (h + 1) * P, :])
                    # v back to natural [s_k, d] layout via PE transpose
                    v_sb = vpool.tile([P, n_st, P], MD, tag="v")
                    for t in range(n_st):
                        tp = aps.tile([P, SQ], MD, tag="scores")
                        nc.tensor.transpose(
                            tp[:, :P], vT[:, t * P:(t + 1) * P], ident[:])
                        nc.vector.tensor_copy(v_sb[:, t, :], tp[:, :P])

                    for b in range(n_sq):
                        ts_here = [t for t in range(n_st)
                                   if (t, b) in block_cls]
                        ps_o = aps.tile([P, SQ], F32, tag="out")
                        ps_row = aps1.tile([P, SQ], F32, tag="row")
                        for i, t in enumerate(ts_here):
                            ps_s = aps.tile([P, SQ], F32, tag="scores")
                            nc.tensor.matmul(
                                ps_s[:],
                                kT[:, t * P:(t + 1) * P],
                                qT[:, b * SQ:(b + 1) * SQ],
                                start=True, stop=True)
                            if block_cls[(t, b)] == "mask":
                                nc.vector.tensor_add(
                                    ps_s[:], ps_s[:],
                                    mtile[:, mask_slot[(t, b)], :])
                            ex = epool.tile([P, SQ], MD, tag="exp")
                            nc.scalar.activation(
                                ex[:], ps_s[:],
                                mybir.ActivationFunctionType.Exp)
                            first, last = i == 0, i == len(ts_here) - 1
                            nc.tensor.matmul(
                                ps_o[:], v_sb[:, t, :], ex[:],
                                start=first, stop=last)
                            # rowsum broadcast to all partitions via the
                            # all-ones stationary operand
                            nc.tensor.matmul(
                                ps_row[:], ones_sq[:], ex[:],
                                start=first, stop=last)
                        recip = rpool.tile([P, SQ], F32, tag="recip")
                        nc.vector.reciprocal(recip[:], ps_row[:])
                        ob = opool.tile([P, SQ], MD, tag="ob")
                        nc.vector.tensor_mul(ob[:], ps_o[:], recip[:])
                        nc.sync.dma_start(
                            gat_in.ap()[h * P:(h + 1) * P,
                                        b * SQ:(b + 1) * SQ], ob[:])

            # =============== phase 3: AllGather context ===============
            nc.gpsimd.collective_compute(
                "AllGather", mybir.AluOpType.bypass,
                replica_groups=[list(range(N_CORES))],
                ins=[gat_in.ap().opt()], outs=[ct.ap().opt()])

            # =============== phase 4: o_proj (column shard) ===============
            with (
                tc.tile_pool(name="op_w", bufs=1) as owpool,
                tc.tile_pool(name="op_ct", bufs=40) as ctpool,
                tc.tile_pool(name="op_stage", bufs=4) as ospool,
                tc.tile_pool(name="op_ps", bufs=4, space="PSUM") as opspool,
            ):
                wo_sb = owpool.tile([P, n_ht, dpc], MD, tag="wo")
                nc.sync.dma_start(wo_sb[:], woT_t[:])
                for sb in range(n_sq):
                    cts = []
                    for t in range(n_ht):
                        c_t = ctpool.tile([P, SQ], MD, tag="ct")
                        nc.sync.dma_start(
                            c_t[:], ct_t[:, t, sb * SQ:(sb + 1) * SQ])
                        cts.append(c_t)
                    for st in range(SQ // P):
                        ps = opspool.tile([P, dpc], F32, tag="op")
                        for t in range(n_ht):
                            nc.tensor.matmul(
                                ps[:],
                                cts[t][:, st * P:(st + 1) * P],
                                wo_sb[:, t, :],
                                start=(t == 0), stop=(t == n_ht - 1))
                        ob = ospool.tile([P, dpc], F32, tag="ostage")
                        nc.scalar.copy(ob[:], ps[:])
                        nc.sync.dma_start(
                            out_cols.ap()[sb * SQ + st * P:
                                          sb * SQ + (st + 1) * P, :], ob[:])

    nc.compile()
    return nc


def _classify_blocks(maskT_np, S):
    """Classify each [128, SQ] scoresT block of the (transposed) mask."""
    cls = {}
    for t in range(S // P):
        rows = maskT_np[t * P:(t + 1) * P]
        for b in range(S // SQ):
            blk = rows[:, b * SQ:(b + 1) * SQ]
            if np.all(blk <= -1e30):
                continue                      # fully masked: skip compute
            if np.all(blk == 0.0):
                cls[(t, b)] = "plain"
            else:
                cls[(t, b)] = "mask"
    return cls


def kernel(hidden_states, attention_mask, w_pack, w_o):
    B, S, H = hidden_states.shape
    assert B == 1 and H == NUM_HEADS * HEAD_DIM
    assert S % (2 * SQ) == 0
    hpc = NUM_HEADS // N_CORES
    dpc = hpc * HEAD_DIM

    xT = np.ascontiguousarray(hidden_states[0].T, dtype=np.float32)
    maskT_np = np.ascontiguousarray(
        np.broadcast_to(attention_mask, (1, 1, S, S))[0, 0].T,
        dtype=np.float32)
    block_cls = _classify_blocks(maskT_np, S)

    key = (S, H, tuple(sorted(block_cls.items())), USE_F32R)
    if key not in _CACHE:
        _CACHE[key] = build(S, H, block_cls, USE_F32R)
    nc = _CACHE[key]

    in_maps = []
    for c in range(N_CORES):
        sl = slice(c * dpc, (c + 1) * dpc)
        wqkv_c = np.concatenate(
            [w_pack[0 * H:1 * H][sl], w_pack[1 * H:2 * H][sl],
             w_pack[2 * H:3 * H][sl]], axis=0)
        in_maps.append({
            "xT": xT,
            "wqkvT": np.ascontiguousarray(wqkv_c.T, dtype=np.float32),
            "maskT": maskT_np,
            "woT": np.ascontiguousarray(w_o[sl].T, dtype=np.float32),
        })

    res = run_bass_kernel_spmd(nc, in_maps, core_ids=list(range(N_CORES)))
    out = np.concatenate(
        [res.results[c]["out_cols"] for c in range(N_CORES)], axis=1)
    return out.reshape(1, S, H).astype(np.float32)


# revision 7
# speedup vs baseline: 1.1583x; 1.1583x over previous
"""Baichuan attention on 8 Trainium2 NeuronCores — tensor-parallel over heads.

Sharding: core c computes heads [4c, 4c+4): its slice of the fused QKV
projection, attention for those heads, then 1/8 of o_proj's output columns
after an AllGather of the per-core context slices (moves 4MB/rank instead of
a 32MB AllReduce of partial sums; mathematically identical to the module's
world_size logic).

Device layout is fully transposed: scores are computed as scoresT[k, q]
blocks so the PE contraction dim (head_dim / s_k) always sits on SBUF
partitions and every matmul has a 512-wide moving operand. Matmul operands
are fp16 (PE 1 cyc/row; 4-byte operands stream at 2 cyc/row) with fp32
accumulation in PSUM — measured end-to-end error vs the fp32 reference is
~4e-4 absmax-relative, on par with the f32r (tf32-like) path.
"""

import numpy as np

import concourse.bacc as bacc
import concourse.mybir as mybir
import concourse.tile as tile
from concourse.bass_utils import run_bass_kernel_spmd
from concourse.masks import make_identity

F32 = mybir.dt.float32

N_CORES = 8
NUM_HEADS = 32
HEAD_DIM = 128
P = 128          # SBUF partitions / PE contraction tile
SQ = 512         # s_q block width (PSUM bank = 512 fp32)
MM_MODE = "f16"  # 'f16' | 'f32r' | 'f32'

_CACHE: dict = {}


def _mm_dtype(mode):
    return {"f16": mybir.dt.float16, "f32r": mybir.dt.float32r,
            "f32": F32}[mode]


def build(S, H, block_cls, mode=MM_MODE):
    """Build the SPMD program. block_cls[(t, b)] = 'plain' | 'mask' for every
    computed scoresT block ([128 s_k] x [SQ s_q]); absent = fully masked, skip.
    """
    MD = _mm_dtype(mode)
    hpc = NUM_HEADS // N_CORES          # heads per core
    dpc = hpc * HEAD_DIM                # per-core slice of the hidden dim
    n_ht = H // P                       # contraction tiles for QKV/o_proj
    n_ot = 3 * dpc // P                 # per-core QKV output tiles
    n_sq = S // SQ                      # s_q blocks
    n_st = S // P                       # s_k tiles
    scale = 1.0 / np.sqrt(np.float32(HEAD_DIM))
    # fp16 x fits in SBUF whole; 4-byte modes need two passes over s
    n_halves = 1 if mybir.dt.size(MD) == 2 else 2
    s_half = S // n_halves
    sb_per_half = s_half // SQ

    nc = bacc.Bacc("TRN2", target_bir_lowering=False, debug=False,
                   num_devices=N_CORES)

    xT = nc.dram_tensor("xT", [H, S], MD, kind="ExternalInput")
    wqkvT = nc.dram_tensor("wqkvT", [H, 3 * dpc], MD, kind="ExternalInput")
    maskT = nc.dram_tensor("maskT", [S, S], F32, kind="ExternalInput")
    woT = nc.dram_tensor("woT", [H, dpc], MD, kind="ExternalInput")
    out_cols = nc.dram_tensor("out_cols", [S, dpc], F32, kind="ExternalOutput")

    qkvT_s = nc.dram_tensor("qkvT_s", [3 * dpc, S], MD)
    gat_in = nc.dram_tensor("gat_in", [dpc, S], MD)
    ct = nc.dram_tensor("ct", [H, S], MD, addr_space="Shared")

    xT_t = xT.ap().rearrange("(t p) s -> p t s", p=P)
    wqkvT_t = wqkvT.ap().rearrange("(t p) o -> p t o", p=P)
    woT_t = woT.ap().rearrange("(t p) j -> p t j", p=P)
    ct_t = ct.ap().rearrange("(t p) s -> p t s", p=P)

    # which mask blocks are needed (shared across heads)
    mask_blocks = sorted({k for k, v in block_cls.items() if v == "mask"})
    mask_slot = {k: i for i, k in enumerate(mask_blocks)}

    with tile.TileContext(nc) as tc:
        # ---- constants (built in f32, rounded to MD via ACT copy) ----
        with tc.tile_pool(name="consts", bufs=1) as cpool:
            ident_f = cpool.tile([P, P], F32, tag="ident_f")
            make_identity(nc, ident_f[:])
            ident = cpool.tile([P, P], MD, tag="ident")
            nc.scalar.copy(ident[:], ident_f[:])
            ones_f = cpool.tile([P, P], F32, tag="ones_f")
            nc.gpsimd.memset(ones_f[:], 1.0)
            ones_sq = cpool.tile([P, P], MD, tag="ones_sq")
            nc.scalar.copy(ones_sq[:], ones_f[:])

            # =============== phase 1: QKV projection ===============
            with (
                tc.tile_pool(name="qkv_sb", bufs=1) as xpool,
                tc.tile_pool(name="qkv_w", bufs=3) as wpool,
                tc.tile_pool(name="qkv_stage", bufs=4) as spool,
                tc.tile_pool(name="qkv_ps", bufs=4, space="PSUM") as pspool,
            ):
                for half in range(n_halves):
                    x_tile = xpool.tile([P, n_ht, s_half], MD, tag="x")
                    for t in range(n_ht):
                        nc.sync.dma_start(
                            x_tile[:, t, :],
                            xT_t[:, t, half * s_half:(half + 1) * s_half])
                    for ot in range(n_ot):
                        w_tile = wpool.tile([P, n_ht, P], MD, tag="w")
                        nc.sync.dma_start(
                            w_tile[:], wqkvT_t[:, :, ot * P:(ot + 1) * P])
                        for sb in range(sb_per_half):
                            ps = pspool.tile([P, SQ], F32, tag="qkv")
                            for t in range(n_ht):
                                nc.tensor.matmul(
                                    ps[:],
                                    w_tile[:, t, :],
                                    x_tile[:, t, sb * SQ:(sb + 1) * SQ],
                                    start=(t == 0), stop=(t == n_ht - 1))
                            st = spool.tile([P, SQ], MD, tag="stage")
                            # fold the softmax scale into q at eviction
                            mul = scale if ot < dpc // P else 1.0
                            nc.scalar.mul(st[:], ps[:], mul)
                            nc.sync.dma_start(
                                qkvT_s.ap()[ot * P:(ot + 1) * P,
                                            half * s_half + sb * SQ:
                                            half * s_half + (sb + 1) * SQ],
                                st[:])

            # =============== phase 2: attention per head ===============
            with (
                tc.tile_pool(name="at_mask", bufs=1) as mpool,
                tc.tile_pool(name="at_qkv", bufs=2) as hpool,
                tc.tile_pool(name="at_v", bufs=2) as vpool,
                tc.tile_pool(name="at_exp", bufs=3) as epool,
                tc.tile_pool(name="at_out", bufs=3) as opool,
                tc.tile_pool(name="at_r", bufs=2) as rpool,
                tc.tile_pool(name="at_ps", bufs=2, space="PSUM") as aps,
                tc.tile_pool(name="at_ps1", bufs=1, space="PSUM") as aps1,
            ):
                if mask_blocks:
                    mtile = mpool.tile([P, len(mask_blocks), SQ], F32,
                                       tag="mask")
                    for (t, b), i in mask_slot.items():
                        nc.sync.dma_start(
                            mtile[:, i, :],
                            maskT.ap()[t * P:(t + 1) * P,
                                       b * SQ:(b + 1) * SQ])

                for h in range(hpc):
                    qT = hpool.tile([P, S], MD, tag="qT")
                    kT = hpool.tile([P, S], MD, tag="kT")
                    vT = hpool.tile([P, S], MD, tag="vT")
                    nc.sync.dma_start(qT[:], qkvT_s.ap()[h * P:(h + 1) * P, :])
                    nc.sync.dma_start(
                        kT[:], qkvT_s.ap()[dpc + h * P:dpc + (h + 1) * P, :])
                    nc.sync.dma_start(
                        vT[:], qkvT_s.ap()[2 * dpc + h * P:
                                           2 * dpc + (h + 1) * P, :])
                    # v back to natural [s_k, d] layout via PE transpose
                    v_sb = vpool.tile([P, n_st, P], MD, tag="v")
                    for t in range(n_st):
                        tp = aps.tile([P, SQ], MD, tag="scores")
                        nc.tensor.transpose(
                            tp[:, :P], vT[:, t * P:(t + 1) * P], ident[:])
                        nc.vector.tensor_copy(v_sb[:, t, :], tp[:, :P])

                    for b in range(n_sq):
                        ts_here = [t for t in range(n_st)
                                   if (t, b) in block_cls]
                        ps_o = aps.tile([P, SQ], F32, tag="out")
                        ps_row = aps1.tile([P, SQ], F32, tag="row")
                        for i, t in enumerate(ts_here):
                            ps_s = aps.tile([P, SQ], F32, tag="scores")
                            nc.tensor.matmul(
                                ps_s[:],
                                kT[:, t * P:(t + 1) * P],
                                qT[:, b * SQ:(b + 1) * SQ],
                                start=True, stop=True)
                            if block_cls[(t, b)] == "mask":
                                nc.vector.tensor_add(
                                    ps_s[:], ps_s[:],
                                    mtile[:, mask_slot[(t, b)], :])
                            ex = epool.tile([P, SQ], MD, tag="exp")
                            nc.scalar.activation(
                                ex[:], ps_s[:],
                                mybir.ActivationFunctionType.Exp)
                            first, last = i == 0, i == len(ts_here) - 1
                            nc.tensor.matmul(
                                ps_o[:], v_sb[:, t, :], ex[:],
                                start=first, stop=last)
                            # rowsum broadcast to all partitions via the
                            # all-ones stationary operand
                            nc.tensor.matmul(
                                ps_row[:], ones_sq[:], ex[:],
                                start=first, stop=last)
                        recip = rpool.tile([P, SQ], F32, tag="recip")
                        nc.vector.reciprocal(recip[:], ps_row[:])
                        ob = opool.tile([P, SQ], MD, tag="ob")
                        nc.vector.tensor_mul(ob[:], ps_o[:], recip[:])
                        nc.sync.dma_start(
                            gat_in.ap()[h * P:(h + 1) * P,
                                        b * SQ:(b + 1) * SQ], ob[:])

            # =============== phase 3: AllGather context ===============
            nc.gpsimd.collective_compute(
                "AllGather", mybir.AluOpType.bypass,
                replica_groups=[list(range(N_CORES))],
                ins=[gat_in.ap().opt()], outs=[ct.ap().opt()])

            # =============== phase 4: o_proj (column shard) ===============
            with (
                tc.tile_pool(name="op_w", bufs=1) as owpool,
                tc.tile_pool(name="op_ct", bufs=40) as ctpool,
                tc.tile_pool(name="op_stage", bufs=4) as ospool,
                tc.tile_pool(name="op_ps", bufs=4, space="PSUM") as opspool,
            ):
                wo_sb = owpool.tile([P, n_ht, dpc], MD, tag="wo")
                nc.sync.dma_start(wo_sb[:], woT_t[:])
                for sb in range(n_sq):
                    cts = []
                    for t in range(n_ht):
                        c_t = ctpool.tile([P, SQ], MD, tag="ct")
                        nc.sync.dma_start(
                            c_t[:], ct_t[:, t, sb * SQ:(sb + 1) * SQ])
                        cts.append(c_t)
                    for st in range(SQ // P):
                        ps = opspool.tile([P, dpc], F32, tag="op")
                        for t in range(n_ht):
                            nc.tensor.matmul(
                                ps[:],
                                cts[t][:, st * P:(st + 1) * P],
                                wo_sb[:, t, :],
                                start=(t == 0), stop=(t == n_ht - 1))
                        ob = ospool.tile([P, dpc], F32, tag="ostage")
                        nc.scalar.copy(ob[:], ps[:])
                        nc.sync.dma_start(
                            out_cols.ap()[sb * SQ + st * P:
                                          sb * SQ + (st + 1) * P, :], ob[:])

    nc.compile()
    return nc


def _classify_blocks(maskT_np, S):
    """Classify each [128, SQ] scoresT block of the (transposed) mask."""
    cls = {}
    for t in range(S // P):
        rows = maskT_np[t * P:(t + 1) * P]
        for b in range(S // SQ):
            blk = rows[:, b * SQ:(b + 1) * SQ]
            if np.all(blk <= -1e30):
                continue                      # fully masked: skip compute
            if np.all(blk == 0.0):
                cls[(t, b)] = "plain"
            else:
                cls[(t, b)] = "mask"
    return cls


def make_in_maps(hidden_states, attention_mask, w_pack, w_o):
    B, S, H = hidden_states.shape
    hpc = NUM_HEADS // N_CORES
    dpc = hpc * HEAD_DIM
    np_md = mybir.dt.np(_mm_dtype(MM_MODE))
    xT = np.ascontiguousarray(hidden_states[0].T).astype(np_md)
    maskT_np = np.ascontiguousarray(
        np.broadcast_to(attention_mask, (1, 1, S, S))[0, 0].T,
        dtype=np.float32)
    in_maps = []
    for c in range(N_CORES):
        sl = slice(c * dpc, (c + 1) * dpc)
        wqkv_c = np.concatenate(
            [w_pack[0 * H:1 * H][sl], w_pack[1 * H:2 * H][sl],
             w_pack[2 * H:3 * H][sl]], axis=0)
        in_maps.append({
            "xT": xT,
            "wqkvT": np.ascontiguousarray(wqkv_c.T).astype(np_md),
            "maskT": maskT_np,
            "woT": np.ascontiguousarray(w_o[sl].T).astype(np_md),
        })
    return in_maps, maskT_np


def kernel(hidden_states, attention_mask, w_pack, w_o):
    B, S, H = hidden_states.shape
    assert B == 1 and H == NUM_HEADS * HEAD_DIM
    assert S % (2 * SQ) == 0

    in_maps, maskT_np = make_in_maps(hidden_states, attention_mask,
                                     w_pack, w_o)
    block_cls = _classify_blocks(maskT_np, S)

    key = (S, H, tuple(sorted(block_cls.items())), MM_MODE)
    if key not in _CACHE:
        _CACHE[key] = build(S, H, block_cls, MM_MODE)
    nc = _CACHE[key]

    res = run_bass_kernel_spmd(nc, in_maps, core_ids=list(range(N_CORES)))
    out = np.concatenate(
        [res.results[c]["out_cols"] for c in range(N_CORES)], axis=1)
    return out.reshape(1, S, H).astype(np.float32)


# revision 8
# speedup vs baseline: 1.2972x; 1.1199x over previous
"""Baichuan attention on 8 Trainium2 NeuronCores — tensor-parallel over heads.

Sharding: core c computes heads [4c, 4c+4): its slice of the fused QKV
projection, attention for those heads, then 1/8 of o_proj's output columns
after an AllGather of the per-core context slices (moves 4MB/rank instead of
a 32MB AllReduce of partial sums; mathematically identical to the module's
world_size logic).

Layout: scores are computed transposed (scoresT[k, q] blocks) so the PE
contraction dim always sits on SBUF partitions and every matmul streams a
512-wide moving operand. Matmul operands are fp16 (1 cyc/row on the PE) with
fp32 PSUM accumulation — measured end-to-end error vs the fp32 reference is
~6e-4 absmax-relative, on par with the f32r (tf32) path. The AllGather is
chunked over four s_q blocks so collective latency and o_proj overlap the
attention of later blocks, keeping the PE stream dense (HAM stays warm).
"""

import numpy as np

import concourse.bacc as bacc
import concourse.mybir as mybir
import concourse.tile as tile
from concourse.bass_utils import run_bass_kernel_spmd

F32 = mybir.dt.float32

N_CORES = 8
NUM_HEADS = 32
HEAD_DIM = 128
P = 128          # SBUF partitions / PE contraction tile
SQ = 512         # s_q block width (PSUM bank = 512 fp32)
MM_MODE = "f16"  # 'f16' | 'f32' (operand dtype for matmuls)

_CACHE: dict = {}


def _mm_dtype(mode):
    return {"f16": mybir.dt.float16, "f32": F32}[mode]


def build(S, H, block_cls, mode=MM_MODE):
    """Build the SPMD program. block_cls[(t, b)] = 'plain' | 'mask' for every
    computed scoresT block ([128 s_k] x [SQ s_q]); absent = fully masked, skip.
    """
    MD = _mm_dtype(mode)
    hpc = NUM_HEADS // N_CORES          # heads per core
    dpc = hpc * HEAD_DIM                # per-core slice of the hidden dim
    n_ht = H // P                       # contraction tiles for QKV/o_proj
    n_qk = 2 * dpc // P                 # q+k output tiles
    n_sq = S // SQ                      # s_q blocks
    n_st = S // P                       # s_k tiles
    scale = 1.0 / np.sqrt(np.float32(HEAD_DIM))
    s_half = S // 2
    sb_per_half = s_half // SQ

    nc = bacc.Bacc("TRN2", target_bir_lowering=False, debug=False,
                   num_devices=N_CORES)

    xT = nc.dram_tensor("xT", [H, S], MD, kind="ExternalInput")
    wqkT = nc.dram_tensor("wqkT", [H, 2 * dpc], MD, kind="ExternalInput")
    wvT = nc.dram_tensor("wvT", [H, dpc], MD, kind="ExternalInput")
    maskT = nc.dram_tensor("maskT", [S, S], F32, kind="ExternalInput")
    woT = nc.dram_tensor("woT", [H, dpc], MD, kind="ExternalInput")
    out_cols = nc.dram_tensor("out_cols", [S, dpc], F32, kind="ExternalOutput")

    qkT_s = nc.dram_tensor("qkT_s", [2 * dpc, S], MD)
    gat_b = [nc.dram_tensor(f"gat_{b}", [dpc, SQ], MD) for b in range(n_sq)]
    ct_b = [nc.dram_tensor(f"ct_{b}", [H, SQ], MD, addr_space="Shared")
            for b in range(n_sq)]

    xT_t = xT.ap().rearrange("(t p) s -> p t s", p=P)
    wqkT_t = wqkT.ap().rearrange("(t p) o -> p t o", p=P)
    wvT_t = wvT.ap().rearrange("(t p) o -> p t o", p=P)
    woT_t = woT.ap().rearrange("(t p) j -> p t j", p=P)
    qkT_r = qkT_s.ap().rearrange("(r p) s -> p r s", p=P)

    mask_blocks = sorted({k for k, v in block_cls.items() if v == "mask"})
    mask_slot = {k: i for i, k in enumerate(mask_blocks)}

    with tile.TileContext(nc) as tc:
        with (
            tc.tile_pool(name="consts", bufs=1) as cpool,
            tc.tile_pool(name="span", bufs=1) as span,
        ):
            ones_f = cpool.tile([P, P], F32, tag="ones_f")
            nc.gpsimd.memset(ones_f[:], 1.0)
            ones_sq = cpool.tile([P, P], MD, tag="ones_sq")
            nc.scalar.copy(ones_sq[:], ones_f[:])

            # v ([s_k, d] natural, all heads) lives in SBUF across phases 1-2
            v_sb = span.tile([P, n_st, dpc], MD, tag="v")

            # =============== phase 1: QKV projection ===============
            # q/k in transposed orientation -> DRAM scratch; v in natural
            # orientation (x stationary, Wv moving) -> resident v_sb.
            with (
                tc.tile_pool(name="qkv_x", bufs=1) as xpool,
                tc.tile_pool(name="qkv_w", bufs=3) as wpool,
                tc.tile_pool(name="qkv_wv", bufs=1) as wvpool,
                tc.tile_pool(name="qkv_stage", bufs=4) as spool,
                tc.tile_pool(name="qkv_ps", bufs=4, space="PSUM") as pspool,
            ):
                wv_sb = wvpool.tile([P, n_ht, dpc], MD, tag="wv")
                nc.sync.dma_start(wv_sb[:], wvT_t[:])
                for half in range(2):
                    x_tile = xpool.tile([P, n_ht, s_half], MD, tag="x")
                    for t in range(n_ht):
                        nc.sync.dma_start(
                            x_tile[:, t, :],
                            xT_t[:, t, half * s_half:(half + 1) * s_half])
                    # v: psum [s=128, dpc] accumulated over h-tiles
                    for sti in range(s_half // P):
                        st_g = half * (s_half // P) + sti
                        ps_v = pspool.tile([P, dpc], F32, tag="qkv")
                        for t in range(n_ht):
                            nc.tensor.matmul(
                                ps_v[:],
                                x_tile[:, t, sti * P:(sti + 1) * P],
                                wv_sb[:, t, :],
                                start=(t == 0), stop=(t == n_ht - 1))
                        nc.vector.tensor_copy(v_sb[:, st_g, :], ps_v[:])
                    # q/k: psum [o=128, s-block] transposed orientation
                    for ot in range(n_qk):
                        w_tile = wpool.tile([P, n_ht, P], MD, tag="w")
                        nc.sync.dma_start(
                            w_tile[:], wqkT_t[:, :, ot * P:(ot + 1) * P])
                        for sb in range(sb_per_half):
                            ps = pspool.tile([P, SQ], F32, tag="qkv")
                            for t in range(n_ht):
                                nc.tensor.matmul(
                                    ps[:],
                                    w_tile[:, t, :],
                                    x_tile[:, t, sb * SQ:(sb + 1) * SQ],
                                    start=(t == 0), stop=(t == n_ht - 1))
                            st = spool.tile([P, SQ], MD, tag="stage")
                            # fold the softmax scale into q at eviction
                            mul = scale if ot < dpc // P else 1.0
                            nc.scalar.mul(st[:], ps[:], mul)
                            nc.sync.dma_start(
                                qkT_s.ap()[ot * P:(ot + 1) * P,
                                           half * s_half + sb * SQ:
                                           half * s_half + (sb + 1) * SQ],
                                st[:])

            # ====== phases 2-4: attention / chunked AllGather / o_proj ======
            with (
                tc.tile_pool(name="at_mask", bufs=1) as mpool,
                tc.tile_pool(name="at_qk", bufs=1) as qkpool,
                tc.tile_pool(name="at_exp", bufs=3) as epool,
                tc.tile_pool(name="at_out", bufs=3) as opool,
                tc.tile_pool(name="at_r", bufs=2) as rpool,
                tc.tile_pool(name="op_w", bufs=1) as owpool,
                tc.tile_pool(name="op_ct", bufs=40) as ctpool,
                tc.tile_pool(name="op_stage", bufs=4) as ospool,
                tc.tile_pool(name="at_ps", bufs=2, space="PSUM") as aps,
                tc.tile_pool(name="op_ps", bufs=2, space="PSUM") as opspool,
            ):
                if mask_blocks:
                    mtile = mpool.tile([P, len(mask_blocks), SQ], F32,
                                       tag="mask")
                    for (t, b), i in mask_slot.items():
                        nc.sync.dma_start(
                            mtile[:, i, :],
                            maskT.ap()[t * P:(t + 1) * P,
                                       b * SQ:(b + 1) * SQ])
                # all q/k heads resident: [p, r, s], r = q0..3,k0..3
                qk_all = qkpool.tile([P, n_qk, S], MD, tag="qk")
                for r in range(n_qk):
                    nc.sync.dma_start(qk_all[:, r, :], qkT_r[:, r, :])
                wo_sb = owpool.tile([P, n_ht, dpc], MD, tag="wo")
                nc.sync.dma_start(wo_sb[:], woT_t[:])

                for b in range(n_sq):
                    ts_here = [t for t in range(n_st) if (t, b) in block_cls]
                    for h in range(hpc):
                        q_sl = qk_all[:, h, b * SQ:(b + 1) * SQ]
                        ps_o = aps.tile([P, SQ], F32, tag="out")
                        ps_row = aps.tile([P, SQ], F32, tag="row")
                        for i, t in enumerate(ts_here):
                            ps_s = aps.tile([P, SQ], F32, tag="scores")
                            nc.tensor.matmul(
                                ps_s[:],
                                qk_all[:, hpc + h, t * P:(t + 1) * P],
                                q_sl, start=True, stop=True)
                            if block_cls[(t, b)] == "mask":
                                nc.vector.tensor_add(
                                    ps_s[:], ps_s[:],
                                    mtile[:, mask_slot[(t, b)], :])
                            ex = epool.tile([P, SQ], MD, tag="exp")
                            nc.scalar.activation(
                                ex[:], ps_s[:],
                                mybir.ActivationFunctionType.Exp)
                            first, last = i == 0, i == len(ts_here) - 1
                            nc.tensor.matmul(
                                ps_o[:], v_sb[:, t, h * P:(h + 1) * P],
                                ex[:], start=first, stop=last)
                            # rowsum broadcast to all partitions via the
                            # all-ones stationary operand
                            nc.tensor.matmul(
                                ps_row[:], ones_sq[:], ex[:],
                                start=first, stop=last)
                        recip = rpool.tile([P, SQ], F32, tag="recip")
                        nc.vector.reciprocal(recip[:], ps_row[:])
                        ob = opool.tile([P, SQ], MD, tag="ob")
                        nc.vector.tensor_mul(ob[:], ps_o[:], recip[:])
                        nc.sync.dma_start(
                            gat_b[b].ap()[h * P:(h + 1) * P, :], ob[:])

                    nc.gpsimd.collective_compute(
                        "AllGather", mybir.AluOpType.bypass,
                        replica_groups=[list(range(N_CORES))],
                        ins=[gat_b[b].ap().opt()], outs=[ct_b[b].ap().opt()])

                    # o_proj for this s_q block (overlaps later blocks)
                    ct_t = ct_b[b].ap().rearrange("(t p) s -> p t s", p=P)
                    cts = []
                    for t in range(n_ht):
                        c_t = ctpool.tile([P, SQ], MD, tag="ct")
                        nc.sync.dma_start(c_t[:], ct_t[:, t, :])
                        cts.append(c_t)
                    for st in range(SQ // P):
                        ps = opspool.tile([P, dpc], F32, tag="op")
                        for t in range(n_ht):
                            nc.tensor.matmul(
                                ps[:],
                                cts[t][:, st * P:(st + 1) * P],
                                wo_sb[:, t, :],
                                start=(t == 0), stop=(t == n_ht - 1))
                        ob = ospool.tile([P, dpc], F32, tag="ostage")
                        nc.scalar.copy(ob[:], ps[:])
                        nc.sync.dma_start(
                            out_cols.ap()[b * SQ + st * P:
                                          b * SQ + (st + 1) * P, :], ob[:])

    nc.compile()
    return nc


def _classify_blocks(maskT_np, S):
    """Classify each [128, SQ] scoresT block of the (transposed) mask."""
    cls = {}
    for t in range(S // P):
        rows = maskT_np[t * P:(t + 1) * P]
        for b in range(S // SQ):
            blk = rows[:, b * SQ:(b + 1) * SQ]
            if np.all(blk <= -1e30):
                continue                      # fully masked: skip compute
            if np.all(blk == 0.0):
                cls[(t, b)] = "plain"
            else:
                cls[(t, b)] = "mask"
    return cls


def make_in_maps(hidden_states, attention_mask, w_pack, w_o):
    B, S, H = hidden_states.shape
    hpc = NUM_HEADS // N_CORES
    dpc = hpc * HEAD_DIM
    np_md = mybir.dt.np(_mm_dtype(MM_MODE))
    xT = np.ascontiguousarray(hidden_states[0].T).astype(np_md)
    maskT_np = np.ascontiguousarray(
        np.broadcast_to(attention_mask, (1, 1, S, S))[0, 0].T,
        dtype=np.float32)
    in_maps = []
    for c in range(N_CORES):
        sl = slice(c * dpc, (c + 1) * dpc)
        wqk_c = np.concatenate(
            [w_pack[0 * H:1 * H][sl], w_pack[1 * H:2 * H][sl]], axis=0)
        in_maps.append({
            "xT": xT,
            "wqkT": np.ascontiguousarray(wqk_c.T).astype(np_md),
            "wvT": np.ascontiguousarray(w_pack[2 * H:3 * H][sl].T
                                        ).astype(np_md),
            "maskT": maskT_np,
            "woT": np.ascontiguousarray(w_o[sl].T).astype(np_md),
        })
    return in_maps, maskT_np


def kernel(hidden_states, attention_mask, w_pack, w_o):
    B, S, H = hidden_states.shape
    assert B == 1 and H == NUM_HEADS * HEAD_DIM
    assert S % (2 * SQ) == 0

    in_maps, maskT_np = make_in_maps(hidden_states, attention_mask,
                                     w_pack, w_o)
    block_cls = _classify_blocks(maskT_np, S)

    key = (S, H, tuple(sorted(block_cls.items())), MM_MODE)
    if key not in _CACHE:
        _CACHE[key] = build(S, H, block_cls, MM_MODE)
    nc = _CACHE[key]

    res = run_bass_kernel_spmd(nc, in_maps, core_ids=list(range(N_CORES)))
    out = np.concatenate(
        [res.results[c]["out_cols"] for c in range(N_CORES)], axis=1)
    return out.reshape(1, S, H).astype(np.float32)


# revision 15
# speedup vs baseline: 1.3521x; 1.0423x over previous
"""Baichuan attention on 8 Trainium2 NeuronCores — tensor-parallel over heads.

Sharding: core c computes heads [4c, 4c+4): its slice of the fused QKV
projection, attention for those heads, then 1/8 of o_proj's output columns
after an AllGather of the per-core context slices (moves 4MB/rank instead of
a 32MB AllReduce of partial sums; mathematically identical to the module's
world_size logic).

Layout: scores are computed transposed (scoresT[k, q] blocks) so the PE
contraction dim always sits on SBUF partitions and every matmul streams a
512-wide moving operand. Matmul operands are fp16 (1 cyc/row on the PE) with
fp32 PSUM accumulation — measured end-to-end error vs the fp32 reference is
~6e-4 absmax-relative, on par with the f32r (tf32) path. The AllGather is
chunked over four s_q blocks so collective latency and o_proj overlap the
attention of later blocks, keeping the PE stream dense (HAM stays warm).
"""

import numpy as np

import concourse.bacc as bacc
import concourse.mybir as mybir
import concourse.tile as tile
from concourse.bass_utils import run_bass_kernel_spmd

F32 = mybir.dt.float32

N_CORES = 8
NUM_HEADS = 32
HEAD_DIM = 128
P = 128          # SBUF partitions / PE contraction tile
SQ = 512         # s_q block width (PSUM bank = 512 fp32)
MM_MODE = "f16"  # 'f16' | 'f32' (operand dtype for matmuls)

_CACHE: dict = {}


def _mm_dtype(mode):
    return {"f16": mybir.dt.float16, "f32": F32}[mode]


def build(S, H, block_cls, mode=MM_MODE):
    """Build the SPMD program. block_cls[(t, b)] = 'plain' | 'mask' for every
    computed scoresT block ([128 s_k] x [SQ s_q]); absent = fully masked, skip.
    """
    MD = _mm_dtype(mode)
    hpc = NUM_HEADS // N_CORES          # heads per core
    dpc = hpc * HEAD_DIM                # per-core slice of the hidden dim
    n_ht = H // P                       # contraction tiles for QKV/o_proj
    n_qk = 2 * dpc // P                 # q+k output tiles
    n_sq = S // SQ                      # s_q blocks
    n_st = S // P                       # s_k tiles
    scale = 1.0 / np.sqrt(np.float32(HEAD_DIM))
    s_half = S // 2
    sb_per_half = s_half // SQ

    nc = bacc.Bacc("TRN2", target_bir_lowering=False, debug=False,
                   num_devices=N_CORES)

    xT = nc.dram_tensor("xT", [H, S], MD, kind="ExternalInput")
    wqkT = nc.dram_tensor("wqkT", [H, 2 * dpc], MD, kind="ExternalInput")
    wvT = nc.dram_tensor("wvT", [H, dpc], MD, kind="ExternalInput")
    maskT = nc.dram_tensor("maskT", [S, S], F32, kind="ExternalInput")
    woT = nc.dram_tensor("woT", [H, dpc], MD, kind="ExternalInput")
    out_cols = nc.dram_tensor("out_cols", [S, dpc], F32, kind="ExternalOutput")

    gat_b = [nc.dram_tensor(f"gat_{b}", [dpc, SQ], MD) for b in range(n_sq)]
    ct_b = [nc.dram_tensor(f"ct_{b}", [H, SQ], MD, addr_space="Shared")
            for b in range(n_sq)]

    xT_t = xT.ap().rearrange("(t p) s -> p t s", p=P)
    wqkT_t = wqkT.ap().rearrange("(t p) o -> p t o", p=P)
    wvT_t = wvT.ap().rearrange("(t p) o -> p t o", p=P)
    woT_t = woT.ap().rearrange("(t p) j -> p t j", p=P)

    # sorted by (b, t) so block b=0's diagonal tiles arrive first
    mask_blocks = sorted({k for k, v in block_cls.items() if v == "mask"},
                         key=lambda k: (k[1], k[0]))
    mask_slot = {k: i for i, k in enumerate(mask_blocks)}

    with tile.TileContext(nc) as tc:
        with (
            tc.tile_pool(name="consts", bufs=1) as cpool,
            tc.tile_pool(name="span", bufs=1) as span,
        ):
            ones_f = cpool.tile([P, P], F32, tag="ones_f")
            nc.gpsimd.memset(ones_f[:], 1.0)
            ones_sq = cpool.tile([P, P], MD, tag="ones_sq")
            nc.scalar.copy(ones_sq[:], ones_f[:])

            # v ([s_k, d] natural, all heads) and q/k (transposed, all heads)
            # live in SBUF across phases 1-2; QKV evictions write them
            # directly (no DRAM bounce)
            v_sb = span.tile([P, n_st, dpc], MD, tag="v")
            qk_all = span.tile([P, n_qk, S], MD, tag="qk")

            # =============== phase 1: QKV projection ===============
            # q/k in transposed orientation -> DRAM scratch; v in natural
            # orientation (x stationary, Wv moving) -> resident v_sb.
            with (
                tc.tile_pool(name="qkv_x", bufs=1) as xpool,
                tc.tile_pool(name="qkv_w", bufs=3) as wpool,
                tc.tile_pool(name="qkv_wv", bufs=1) as wvpool,
                tc.tile_pool(name="qkv_ps", bufs=4, space="PSUM") as pspool,
            ):
                wv_sb = wvpool.tile([P, n_ht, dpc], MD, tag="wv")
                nc.sync.dma_start(wv_sb[:], wvT_t[:])
                for half in range(2):
                    x_tile = xpool.tile([P, n_ht, s_half], MD, tag="x")
                    for t in range(n_ht):
                        nc.sync.dma_start(
                            x_tile[:, t, :],
                            xT_t[:, t, half * s_half:(half + 1) * s_half])
                    # v: psum [s=128, dpc] accumulated over h-tiles
                    for sti in range(s_half // P):
                        st_g = half * (s_half // P) + sti
                        ps_v = pspool.tile([P, dpc], F32, tag="qkv")
                        for t in range(n_ht):
                            nc.tensor.matmul(
                                ps_v[:],
                                x_tile[:, t, sti * P:(sti + 1) * P],
                                wv_sb[:, t, :],
                                start=(t == 0), stop=(t == n_ht - 1))
                        nc.vector.tensor_copy(v_sb[:, st_g, :], ps_v[:])
                    # q/k: psum [o=128, s-block] transposed orientation
                    for ot in range(n_qk):
                        w_tile = wpool.tile([P, n_ht, P], MD, tag="w")
                        nc.sync.dma_start(
                            w_tile[:], wqkT_t[:, :, ot * P:(ot + 1) * P])
                        for sb in range(sb_per_half):
                            ps = pspool.tile([P, SQ], F32, tag="qkv")
                            for t in range(n_ht):
                                nc.tensor.matmul(
                                    ps[:],
                                    w_tile[:, t, :],
                                    x_tile[:, t, sb * SQ:(sb + 1) * SQ],
                                    start=(t == 0), stop=(t == n_ht - 1))
                            # fold the softmax scale into q at eviction;
                            # write straight into the resident qk tile
                            mul = scale if ot < dpc // P else 1.0
                            lo = half * s_half + sb * SQ
                            nc.scalar.mul(qk_all[:, ot, lo:lo + SQ],
                                          ps[:], mul)

            # ====== phases 2-4: attention / chunked AllGather / o_proj ======
            with (
                tc.tile_pool(name="at_mask", bufs=1) as mpool,
                tc.tile_pool(name="at_exp", bufs=3) as epool,
                tc.tile_pool(name="at_out", bufs=3) as opool,
                tc.tile_pool(name="at_r", bufs=2) as rpool,
                tc.tile_pool(name="op_w", bufs=1) as owpool,
                tc.tile_pool(name="op_ct", bufs=40) as ctpool,
                tc.tile_pool(name="op_stage", bufs=4) as ospool,
                tc.tile_pool(name="at_ps", bufs=2, space="PSUM") as aps,
                tc.tile_pool(name="op_ps", bufs=2, space="PSUM") as opspool,
            ):
                if mask_blocks:
                    mtile = mpool.tile([P, len(mask_blocks), SQ], F32,
                                       tag="mask")
                    for (t, b), i in mask_slot.items():
                        nc.sync.dma_start(
                            mtile[:, i, :],
                            maskT.ap()[t * P:(t + 1) * P,
                                       b * SQ:(b + 1) * SQ])
                wo_sb = owpool.tile([P, n_ht, dpc], MD, tag="wo")
                nc.sync.dma_start(wo_sb[:], woT_t[:])

                for b in range(n_sq):
                    ts_here = [t for t in range(n_st) if (t, b) in block_cls]
                    for h in range(hpc):
                        q_sl = qk_all[:, h, b * SQ:(b + 1) * SQ]
                        ps_o = aps.tile([P, SQ], F32, tag="out")
                        ps_row = aps.tile([P, SQ], F32, tag="row")
                        for i, t in enumerate(ts_here):
                            ps_s = aps.tile([P, SQ], F32, tag="scores")
                            nc.tensor.matmul(
                                ps_s[:],
                                qk_all[:, hpc + h, t * P:(t + 1) * P],
                                q_sl, start=True, stop=True)
                            if block_cls[(t, b)] == "mask":
                                nc.vector.tensor_add(
                                    ps_s[:], ps_s[:],
                                    mtile[:, mask_slot[(t, b)], :])
                            ex = epool.tile([P, SQ], MD, tag="exp")
                            nc.scalar.activation(
                                ex[:], ps_s[:],
                                mybir.ActivationFunctionType.Exp)
                            first, last = i == 0, i == len(ts_here) - 1
                            nc.tensor.matmul(
                                ps_o[:], v_sb[:, t, h * P:(h + 1) * P],
                                ex[:], start=first, stop=last)
                            # rowsum broadcast to all partitions via the
                            # all-ones stationary operand
                            nc.tensor.matmul(
                                ps_row[:], ones_sq[:], ex[:],
                                start=first, stop=last)
                        recip = rpool.tile([P, SQ], F32, tag="recip")
                        nc.vector.reciprocal(recip[:], ps_row[:])
                        ob = opool.tile([P, SQ], MD, tag="ob")
                        nc.vector.tensor_mul(ob[:], ps_o[:], recip[:])
                        nc.sync.dma_start(
                            gat_b[b].ap()[h * P:(h + 1) * P, :], ob[:])

                    nc.gpsimd.collective_compute(
                        "AllGather", mybir.AluOpType.bypass,
                        replica_groups=[list(range(N_CORES))],
                        ins=[gat_b[b].ap().opt()], outs=[ct_b[b].ap().opt()])

                    # o_proj for this s_q block (overlaps later blocks)
                    ct_t = ct_b[b].ap().rearrange("(t p) s -> p t s", p=P)
                    cts = []
                    for t in range(n_ht):
                        c_t = ctpool.tile([P, SQ], MD, tag="ct")
                        nc.sync.dma_start(c_t[:], ct_t[:, t, :])
                        cts.append(c_t)
                    for st in range(SQ // P):
                        ps = opspool.tile([P, dpc], F32, tag="op")
                        for t in range(n_ht):
                            nc.tensor.matmul(
                                ps[:],
                                cts[t][:, st * P:(st + 1) * P],
                                wo_sb[:, t, :],
                                start=(t == 0), stop=(t == n_ht - 1))
                        ob = ospool.tile([P, dpc], F32, tag="ostage")
                        nc.scalar.copy(ob[:], ps[:])
                        nc.sync.dma_start(
                            out_cols.ap()[b * SQ + st * P:
                                          b * SQ + (st + 1) * P, :], ob[:])

    nc.compile()
    return nc


def _classify_blocks(maskT_np, S):
    """Classify each [128, SQ] scoresT block of the (transposed) mask."""
    cls = {}
    for t in range(S // P):
        rows = maskT_np[t * P:(t + 1) * P]
        for b in range(S // SQ):
            blk = rows[:, b * SQ:(b + 1) * SQ]
            if np.all(blk <= -1e30):
                continue                      # fully masked: skip compute
            if np.all(blk == 0.0):
                cls[(t, b)] = "plain"
            else:
                cls[(t, b)] = "mask"
    return cls


def make_in_maps(hidden_states, attention_mask, w_pack, w_o):
    B, S, H = hidden_states.shape
    hpc = NUM_HEADS // N_CORES
    dpc = hpc * HEAD_DIM
    np_md = mybir.dt.np(_mm_dtype(MM_MODE))
    xT = np.ascontiguousarray(hidden_states[0].T).astype(np_md)
    maskT_np = np.ascontiguousarray(
        np.broadcast_to(attention_mask, (1, 1, S, S))[0, 0].T,
        dtype=np.float32)
    in_maps = []
    for c in range(N_CORES):
        sl = slice(c * dpc, (c + 1) * dpc)
        wqk_c = np.concatenate(
            [w_pack[0 * H:1 * H][sl], w_pack[1 * H:2 * H][sl]], axis=0)
        in_maps.append({
            "xT": xT,
            "wqkT": np.ascontiguousarray(wqk_c.T).astype(np_md),
            "wvT": np.ascontiguousarray(w_pack[2 * H:3 * H][sl].T
                                        ).astype(np_md),
            "maskT": maskT_np,
            "woT": np.ascontiguousarray(w_o[sl].T).astype(np_md),
        })
    return in_maps, maskT_np


def kernel(hidden_states, attention_mask, w_pack, w_o):
    B, S, H = hidden_states.shape
    assert B == 1 and H == NUM_HEADS * HEAD_DIM
    assert S % (2 * SQ) == 0

    in_maps, maskT_np = make_in_maps(hidden_states, attention_mask,
                                     w_pack, w_o)
    block_cls = _classify_blocks(maskT_np, S)

    key = (S, H, tuple(sorted(block_cls.items())), MM_MODE)
    if key not in _CACHE:
        _CACHE[key] = build(S, H, block_cls, MM_MODE)
    nc = _CACHE[key]

    res = run_bass_kernel_spmd(nc, in_maps, core_ids=list(range(N_CORES)))
    out = np.concatenate(
        [res.results[c]["out_cols"] for c in range(N_CORES)], axis=1)
    return out.reshape(1, S, H).astype(np.float32)
